# revision 8
# baseline (speedup 1.0000x reference)
"""Block-sparse attention (block-diagonal mask, full-row softmax) on 8 trn2 cores.

Reference semantics (B=1, H=16, S=4096, D=64, BLOCK=64):
    scores  = (Q @ K^T) / 8                     [S, S] per head
    scores *= blockdiag_mask                    (off-block -> 0, NOT -inf)
    weights = softmax(scores, axis=-1)          (over the FULL row)
    out     = weights @ V

Off-block entries contribute exp(0)=1 to the softmax, so for row q in
block b:
    num_q   = sum_{k in b} (e_qk - 1) v_k + V_total
    denom_q = sum_{k in b} e_qk - 64 + S
    out_q   = num_q / denom_q
Only the diagonal 64x64 blocks are ever materialized.

Sharding: 16 heads over 8 cores -> 2 heads/core, no cross-core comms.

Per-core pipeline:
  - Q/K fp32 loads on the sync HWDGE ring; V loads via gpsimd SWDGE with
    inline fp32->bf16 cast (third descriptor stream, no DVE cast);
    stores on the scalar HWDGE ring. Both heads' DMAs are issued up
    front (tiles double-buffered) so there is no inter-head stall.
  - Q/K fp32->bf16 casts on DVE (tensor_scalar mul-by-1 for the 2x
    dual-port mode).
  - mask-row trick: rows 64:66 of the staging tiles add -M^2 to
    cross-block scores so exp underflows to exact 0.
  - quad batching: 8 PE transposes land in ONE PSUM bank ([64, 8, 128]),
    one DVE copy stages them; 4 scores matmuls share one PSUM bank
    (start= only on the first clears it), one ACT exp + one DVE
    broadcast-subtract (E-1) cover 4 chunks; 4 po matmuls + one rank-1
    (+[Vtot|S]) accumulate into one bank; one batched reciprocal and one
    broadcast multiply finish 4 chunks.
"""

import numpy as np

H, S, D = 16, 4096, 64
HPC = 2  # heads per core
NCORES = 8
CHUNK = 128
NCHUNK = S // CHUNK  # 32
NQUAD = NCHUNK // 4  # 8
NSLAB = 4
SLABC = NCHUNK // NSLAB  # 8 chunks per slab
SCALE = 0.125  # 1/sqrt(D)
MASK_M = 64.0  # M^2*SCALE = 512: exp underflows to exact 0

_CACHE = {}


def _build_bass():
    import concourse.bass as bass
    import concourse.bacc as bacc
    import concourse.tile as tile
    from concourse import mybir
    from concourse.masks import make_identity

    f32 = mybir.dt.float32
    bf16 = mybir.dt.bfloat16
    EXP = mybir.ActivationFunctionType.Exp

    nc = bacc.Bacc(
        "TRN2", target_bir_lowering=False, debug=False, num_devices=NCORES
    )
    q_d = nc.dram_tensor("query", [HPC, S, D], f32, kind="ExternalInput")
    k_d = nc.dram_tensor("key", [HPC, S, D], f32, kind="ExternalInput")
    v_d = nc.dram_tensor("value", [HPC, S, D], f32, kind="ExternalInput")
    o_d = nc.dram_tensor("out", [HPC, S, D], f32, kind="ExternalOutput")

    NT = 3  # fixed transpose-staging tiles (mask rows written once)

    with tile.TileContext(nc) as tc:
        with (
            tc.tile_pool(name="consts", bufs=1) as consts,
            tc.tile_pool(name="heads", bufs=2) as heads,
            tc.tile_pool(name="work", bufs=4) as work,
            tc.tile_pool(name="norm", bufs=4) as norm,
            tc.tile_pool(name="vt", bufs=2) as vtp,
            tc.tile_pool(name="ps_t", bufs=2, space="PSUM") as ps_t,
            tc.tile_pool(name="ps_s", bufs=2, space="PSUM") as ps_s,
            tc.tile_pool(name="ps_o", bufs=2, space="PSUM") as ps_o,
            tc.tile_pool(name="ps_v", bufs=1, space="PSUM") as ps_v,
            tc.tile_pool(name="ps_w", bufs=1, space="PSUM") as ps_w,
        ):
            identb = consts.tile([128, 128], bf16, tag="identb")
            make_identity(nc, identb)
            # PE warmup: ~3.5us of back-to-back matmuls during the initial
            # DMA wait so the HAM clock-gate releases (K=8/8) before real
            # work starts. Transposes don't count as PE activity for HAM.
            warm = ps_w.tile([128, 128], f32, tag="warm")
            for _ in range(12):
                nc.tensor.matmul(warm, identb, identb, start=True, stop=True)
            ones_col = consts.tile([128, 1], bf16, tag="ones_col")
            nc.gpsimd.memset(ones_col, 1.0)
            ones_row = consts.tile([1, 128], bf16, tag="ones_row")
            nc.gpsimd.memset(ones_row, 1.0)

            # Block-diagonal +1 (subtracted from E on DVE)
            blkdiag = consts.tile([128, 128], bf16, tag="blkdiag")
            nc.gpsimd.memset(blkdiag, 0.0)
            nc.gpsimd.memset(blkdiag[0:64, 0:64], 1.0)
            nc.gpsimd.memset(blkdiag[64:128, 64:128], 1.0)

            # Fixed transpose-staging tiles [66, 8, 128] bf16 per quad:
            # groups [Q c0..c3 | K c0..c3], rows 64:66 = mask rows
            # (written once):
            #   Q side: -M where (r + jb) == 1   (jb = 64-col parity)
            #   K side: +M where  r == jb
            tsbs = []
            for i in range(NT):
                t = consts.tile([66, 8, 128], bf16, tag=f"tsb{i}")
                nc.gpsimd.memset(t[64:66, :, :], 0.0)
                nc.gpsimd.affine_select(
                    out=t[64:66, 0:4, :].rearrange("p w (b j) -> p w b j", b=2),
                    in_=t[64:66, 0:4, :].rearrange("p w (b j) -> p w b j", b=2),
                    compare_op=mybir.AluOpType.not_equal,
                    fill=-MASK_M,
                    base=-1,
                    pattern=[[0, 4], [1, 2], [0, 64]],
                    channel_multiplier=1,
                )
                nc.gpsimd.affine_select(
                    out=t[64:66, 4:8, :].rearrange("p w (b j) -> p w b j", b=2),
                    in_=t[64:66, 4:8, :].rearrange("p w (b j) -> p w b j", b=2),
                    compare_op=mybir.AluOpType.not_equal,
                    fill=MASK_M,
                    base=0,
                    pattern=[[0, 4], [-1, 2], [0, 64]],
                    channel_multiplier=1,
                )
                tsbs.append(t)

            for h in range(HPC):
                qh = heads.tile([128, NCHUNK, D], f32, tag="qh")
                kh = heads.tile([128, NCHUNK, D], f32, tag="kh")
                vh = heads.tile([128, NCHUNK, D], f32, tag="vh")
                oh = heads.tile([128, NCHUNK, D], f32, tag="oh")
                qhb = heads.tile([128, NCHUNK, D], bf16, tag="qhb")
                khb = heads.tile([128, NCHUNK, D], bf16, tag="khb")
                vhb = heads.tile([128, NCHUNK, D + 1], bf16, tag="vhb")

                def slab_dma(eng, dst, src, s):
                    eng.dma_start(
                        out=dst[:, s * SLABC : (s + 1) * SLABC, :],
                        in_=src.rearrange("(c p) d -> p c d", p=128)[
                            :, s * SLABC : (s + 1) * SLABC, :
                        ],
                    )

                # V f32 on the scalar ring (idle early); Q/K on sync.
                for s in range(NSLAB):
                    slab_dma(nc.scalar, vh, v_d[h], s)
                for s in range(NSLAB):
                    slab_dma(nc.sync, qh, q_d[h], s)
                    slab_dma(nc.sync, kh, k_d[h], s)

                nc.vector.memset(vhb[:, :, D : D + 1], 1.0)

                # V slab casts on ACT; Q/K slab casts on DVE
                for s in range(NSLAB):
                    sl = slice(s * SLABC, (s + 1) * SLABC)
                    nc.scalar.copy(out=vhb[:, sl, 0:D], in_=vh[:, sl, :])
                    nc.vector.tensor_scalar_mul(qhb[:, sl, :], qh[:, sl, :], 1.0)
                    nc.vector.tensor_scalar_mul(khb[:, sl, :], kh[:, sl, :], 1.0)

                # V_total colsum: accumulate 8 half-slab matmuls into one
                # [1, 4, 65] PSUM window; DVE tree-add -> vtxb [1, 65]
                # (col D = S = 4096 exactly since vhb col D is all-ones);
                # broadcast-copy to vtx4 [1, 4, 65] for the rank-1 MMs.
                vt_ps = ps_v.tile([1, 4, D + 1], f32, tag="vt_ps")
                for s in range(2 * NSLAB):
                    nc.tensor.matmul(
                        vt_ps,
                        ones_col,
                        vhb[:, 4 * s : 4 * (s + 1), :],
                        start=(s == 0),
                        stop=(s == 2 * NSLAB - 1),
                    )
                vt4 = vtp.tile([1, 4, D + 1], f32, tag="vt4")
                nc.vector.tensor_copy(out=vt4, in_=vt_ps)
                vt2 = vtp.tile([1, 2, D + 1], f32, tag="vt2")
                nc.vector.tensor_add(vt2, vt4[:, 0:2, :], vt4[:, 2:4, :])
                vtxb = vtp.tile([1, D + 1], bf16, tag="vtxb")
                nc.vector.tensor_add(vtxb, vt2[:, 0, :], vt2[:, 1, :])
                vtx4 = vtp.tile([1, 4, D + 1], bf16, tag="vtx4")
                nc.vector.tensor_copy(
                    out=vtx4,
                    in_=vtxb[:].unsqueeze(1).broadcast_to((1, 4, D + 1)),
                )

                for g in range(NQUAD):
                    c0 = 4 * g
                    # 8 transposes -> one PSUM bank [64, Q c0..c3 | K c0..c3, 128]
                    pt = ps_t.tile([64, 8, 128], bf16, tag="pt")
                    for qi in range(4):
                        nc.tensor.transpose(
                            pt[:, qi, :], qhb[:, c0 + qi, :], identb
                        )
                    for qi in range(4):
                        nc.tensor.transpose(
                            pt[:, 4 + qi, :], khb[:, c0 + qi, :], identb
                        )
                    tsb = tsbs[g % NT]
                    nc.vector.tensor_copy(out=tsb[0:64, :, :], in_=pt)

                    # 4 scores matmuls -> one PSUM bank
                    pss = ps_s.tile([128, 4, 128], f32, tag="pss")
                    for qi in range(4):
                        nc.tensor.matmul(
                            pss[:, qi, :],
                            tsb[:, 4 + qi, :],
                            tsb[:, qi, :],
                            start=(qi == 0),
                            stop=(qi == 3),
                        )

                    # E^T = exp(S^T/8) for 4 chunks in one ACT op
                    et = work.tile([128, 4, 128], bf16, tag="et")
                    nc.scalar.activation(out=et, in_=pss, func=EXP, scale=SCALE)
                    # E^T - blockdiag(1): one DVE op, broadcast in1
                    etm = work.tile([128, 4, 128], bf16, tag="etm")
                    nc.vector.tensor_sub(
                        etm,
                        et,
                        blkdiag[:].unsqueeze(1).broadcast_to((128, 4, 128)),
                    )

                    # num|denom: po = (E-1)^T @ [V|1] + ones x [Vtot|S]
                    po = ps_o.tile([128, 4, D + 1], f32, tag="po")
                    for qi in range(4):
                        nc.tensor.matmul(
                            po[:, qi, :],
                            etm[:, qi, :],
                            vhb[:, c0 + qi, :],
                            start=(qi == 0),
                            stop=False,
                        )
                    nc.tensor.matmul(po, ones_row, vtx4, start=False, stop=True)

                    # rcp = 1/denom for 4 chunks; out = num * rcp
                    rr = norm.tile([128, 4], f32, tag="rr")
                    nc.vector.reciprocal(out=rr, in_=po[:, :, D])
                    tqn = norm.tile([128, 4, D], f32, tag="tqn")
                    nc.vector.tensor_copy(out=tqn, in_=po[:, :, 0:D])
                    nc.gpsimd.tensor_mul(
                        oh[:, c0 : c0 + 4, :],
                        tqn,
                        rr[:].unsqueeze(2).broadcast_to((128, 4, D)),
                    )

                # stores on the scalar HWDGE ring, per quarter so they
                # drain during compute
                for quarter in range(4):
                    hs = slice(
                        quarter * (NCHUNK // 4), (quarter + 1) * (NCHUNK // 4)
                    )
                    nc.scalar.dma_start(
                        out=o_d[h].rearrange("(c p) d -> p c d", p=128)[:, hs, :],
                        in_=oh[:, hs, :],
                    )

    nc.compile()
    return nc


def _get_compiled():
    if "nc" not in _CACHE:
        _CACHE["nc"] = _build_bass()
    return _CACHE["nc"]


def make_in_maps(query, key, value):
    q = np.ascontiguousarray(np.asarray(query).reshape(H, S, D), dtype=np.float32)
    k = np.ascontiguousarray(np.asarray(key).reshape(H, S, D), dtype=np.float32)
    v = np.ascontiguousarray(np.asarray(value).reshape(H, S, D), dtype=np.float32)
    in_maps = []
    for i in range(NCORES):
        sl = slice(i * HPC, (i + 1) * HPC)
        in_maps.append(
            {
                "query": np.ascontiguousarray(q[sl]),
                "key": np.ascontiguousarray(k[sl]),
                "value": np.ascontiguousarray(v[sl]),
            }
        )
    return in_maps


def run_spmd(in_maps, **kwargs):
    from concourse.bass_utils import run_bass_kernel_spmd

    nc = _get_compiled()
    return run_bass_kernel_spmd(nc, in_maps, core_ids=list(range(NCORES)), **kwargs)


def assemble(res):
    outs = [res.results[i]["out"] for i in range(NCORES)]
    return np.concatenate(outs, axis=0).reshape(1, H, S, D).astype(np.float32)


def kernel(query: np.ndarray, key: np.ndarray, value: np.ndarray) -> np.ndarray:
    return assemble(run_spmd(make_in_maps(query, key, value)))
